# revision 46
# baseline (speedup 1.0000x reference)
"""DeepSeek-MoE layer on 8 TRN2 NeuronCores.

Strategy (expert-parallel, host-side dispatch):
  - Router (x @ gate_w.T, softmax, top-2) computed on host — it *is* the
    sharding decision (~0.02% of total FLOPs).
  - Core c computes routed expert c's SwiGLU FFN over the tokens routed to
    it (gathered+padded to a fixed capacity), plus a 512-token chunk of
    shared expert c//4 (each shared expert covers all 2048 tokens, split
    over 4 cores).
  - All matmuls in bf16 (fp32 PSUM accumulation). Combine weights /
    scatter-add applied on host in fp32.

Device kernel layout:
  - Tokens live on the matmul free axis (x stored transposed [H, C]).
  - Weights are streamed as 768KB "4-mi chunks" pre-packed on the host
    into their exact SBUF image (gate/up j-outer), ordered in
    consumption order on the sync HWDGE ring with no phase barriers.
  - The PE powers up in the 1.2GHz mid-pstate and drops back after ~2us
    idle; ~48 dummy 128-col matmuls keep it busy from ~7us so the real
    stream starts at the full 2.4GHz (worth ~3-5us).
  - Shared-expert job runs FIRST: its 512-col tile demands weight bytes
    at ~0.14MB/us vs ~0.35MB/us HBM supply, so the ramp is never
    DMA-paced; the routed job (288+256 col-tiles) follows.
  - Fully fused gate/up loop per 128-row slice mi of I: 6 gate MMs, 6 up
    MMs (PSUM), silu (ACT), mul->bf16 (DVE); down-proj is bank-major (24
    MMs per output bank) so each bank's PSUM->SBUF copy + store overlaps
    the next bank's matmuls and only a half-width copy trails the last
    matmul.
  - dma_start costs ~590ns of serialized DIRECT2D enqueue on the issuing
    sequencer: x tensors ride as single rearranged descriptors (xs ahead
    of the weight pieces, xr queued behind job1's 14MB so it lands ~48us
    in, off the HBM-saturated ramp).
  - Output store copies alternate ACT/DVE half-width, stores alternate
    both HWDGE rings.
"""
import os
import sys
import types

import numpy as np
import ml_dtypes

import concourse.bass as bass
import concourse.tile as tile
import concourse.mybir as mybir
from concourse import bacc
from concourse.bass_utils import run_bass_kernel_spmd

# ---- problem constants (DeepSeekMoE: B=2,S=1024,H=768,I=3072,E=8,NS=2,k=2) --
H = 768          # hidden
I = 3072         # intermediate
E = 8            # routed experts
NS = 2           # shared experts
TOP_K = 2
N_CORES = 8
KH = H // 128    # 6 k-tiles over H
KI = I // 128    # 24 mi-tiles over I
NCH = KI // 4    # 6 weight chunk-groups (4 mi each)
CS = 2048 * NS // N_CORES  # shared-expert tokens per core = 512

BF16 = mybir.dt.bfloat16
F32 = mybir.dt.float32
_bf = ml_dtypes.bfloat16


def _install_ntff_hook():
    """Provide antenv.axon_hooks (missing on this image) so trace=True works."""
    if "antenv.axon_hooks" in sys.modules:
        return
    try:
        from trn_agent_boot.trn_boot import _ntff_profile_via_ctypes
        hook = _ntff_profile_via_ctypes("/opt/axon/libaxon_pjrt.so")
    except Exception:
        hook = None
    mod = types.ModuleType("antenv.axon_hooks")
    mod.get_axon_ntff_profile_hook = lambda: hook
    sys.modules["antenv.axon_hooks"] = mod


def _col_tiles(c):
    if c <= 512:
        return [(0, c)]
    half = (c // 2 + 31) // 32 * 32
    return [(0, half), (half, c - half)]


def _ffn_job(nc, wpool, hpool, sgpool, gupool, ypool, ystage,
             x_sb, wchunks, base, y_dram, n_tiles, first=False,
             x_hook=None, x_hook2=None, after_w_hook=None, last=False):
    """One SwiGLU FFN: y = (silu(x Wg) * (x Wu)) Wd for one expert.

    wchunks[base + 3c + {0,1,2}] are the gate/up/down weight chunks for
    mi-group c, pre-packed on host as the exact [128, 3072] SBUF image
    (gate/up images are j-outer so a single j-slice is contiguous).
    """
    gu_t = {}
    wd_t = {}
    for c in range(NCH):
        row_g = base + 3 * c + 0
        row_u = base + 3 * c + 1
        tg = wpool.tile([128, 4, KH, 128], BF16, tag="w")
        tu = wpool.tile([128, 4, KH, 128], BF16, tag="w")
        if first and c == 0:
            # x first, then g0/u0 split into 192KB j-pieces in
            # consumption order: the first matmul is gated on x plus one
            # 192KB piece instead of a whole 768KB chunk
            if x_hook is not None:
                x_hook()
            for j in range(4):
                nc.sync.dma_start(
                    out=tg[:, j], in_=wchunks[row_g, :, j * 768:(j + 1) * 768]
                    .rearrange("p (k m) -> p k m", k=KH))
                if j == 0 and x_hook2 is not None:
                    x_hook2()
                nc.sync.dma_start(
                    out=tu[:, j], in_=wchunks[row_u, :, j * 768:(j + 1) * 768]
                    .rearrange("p (k m) -> p k m", k=KH))
        else:
            nc.sync.dma_start(out=tg, in_=wchunks[row_g, :, :]
                              .rearrange("p (j k m) -> p j k m", j=4, k=KH))
            nc.sync.dma_start(out=tu, in_=wchunks[row_u, :, :]
                              .rearrange("p (j k m) -> p j k m", j=4, k=KH))
        gu_t[c] = (tg, tu)
    for c in range(NCH):
        td = wpool.tile([128, 4, H], BF16, tag="w")
        nc.sync.dma_start(out=td, in_=wchunks[base + 3 * c + 2, :, :]
                          .rearrange("p (j i) -> p j i", j=4))
        wd_t[c] = td
    if after_w_hook is not None:
        after_w_hook()  # bulk x loads queue here, behind this job's chunks

    for ti, (n0, nsz) in enumerate(n_tiles):
        # gate/up + silu*mul for all 24 mi (4 PSUM banks -> the silu/mul
        # round-trip never stalls the next mi's matmuls)
        hs = {}
        for c in range(NCH):
            tg, tu = gu_t[c]
            for j in range(4):
                g = gupool.tile([128, 512], F32, tag="gu")
                u = gupool.tile([128, 512], F32, tag="gu")
                for k in range(KH):
                    nc.tensor.matmul(
                        g[:, :nsz], tg[:, j, k, :], x_sb[:, k, n0:n0 + nsz],
                        start=(k == 0), stop=(k == KH - 1))
                for k in range(KH):
                    nc.tensor.matmul(
                        u[:, :nsz], tu[:, j, k, :], x_sb[:, k, n0:n0 + nsz],
                        start=(k == 0), stop=(k == KH - 1))
                sg = sgpool.tile([128, 512], F32, tag="sg")
                nc.scalar.activation(sg[:, :nsz], g[:, :nsz],
                                     mybir.ActivationFunctionType.Silu)
                h = hpool.tile([128, 512], BF16, tag="h")
                nc.vector.tensor_mul(h[:, :nsz], sg[:, :nsz], u[:, :nsz])
                hs[4 * c + j] = h
        # down proj bank-major (every bank needs all 24 hs tiles anyway):
        # bank hj's copy+store overlaps bank hj+1's matmuls, so only the
        # final bank's half-width copies trail the last matmul
        for hj in range(KH):
            yb = ypool.tile([128, 512], F32, tag="y", name=f"y{hj}")
            h0 = nsz // 2
            # half-width chains only pay off when each half stays above
            # the ~97ns LDWEIGHTS floor (>=240 cols per half)
            final = (last and ti == len(n_tiles) - 1 and hj == KH - 1
                     and nsz >= 480)
            if final:
                # very last bank: two independent half-width accumulation
                # chains so the first half's copy+store overlaps the
                # second half's matmuls — only a half-width copy trails
                # the kernel's last matmul
                for mi in range(KI):
                    nc.tensor.matmul(
                        yb[:, :h0],
                        wd_t[mi // 4][:, mi % 4, hj * 128:(hj + 1) * 128],
                        hs[mi][:, :h0],
                        start=(mi == 0), stop=(mi == KI - 1))
                for mi in range(KI):
                    nc.tensor.matmul(
                        yb[:, h0:nsz],
                        wd_t[mi // 4][:, mi % 4, hj * 128:(hj + 1) * 128],
                        hs[mi][:, h0:nsz],
                        start=(mi == 0), stop=(mi == KI - 1))
            else:
                for mi in range(KI):
                    nc.tensor.matmul(
                        yb[:, :nsz],
                        wd_t[mi // 4][:, mi % 4, hj * 128:(hj + 1) * 128],
                        hs[mi][:, :nsz],
                        start=(mi == 0), stop=(mi == KI - 1))
            # two half-width copies on ACT+DVE, stores on both HWDGE rings
            yst = ystage.tile([128, 512], BF16, tag="yst")
            nc.scalar.copy(yst[:, :h0], yb[:, :h0])
            nc.vector.tensor_copy(yst[:, h0:nsz], yb[:, h0:nsz])
            nc.scalar.dma_start(
                out=y_dram[hj * 128:(hj + 1) * 128, n0:n0 + h0],
                in_=yst[:, :h0])
            nc.sync.dma_start(
                out=y_dram[hj * 128:(hj + 1) * 128, n0 + h0:n0 + nsz],
                in_=yst[:, h0:nsz])


def build_nc(cr):
    """Build the SPMD program. cr = routed-token capacity (multiple of 32)."""
    nc = bacc.Bacc(None, target_bir_lowering=False)
    xr = nc.dram_tensor("xr", [H, cr], BF16, kind="ExternalInput")
    xs = nc.dram_tensor("xs", [H, CS], BF16, kind="ExternalInput")
    wch = nc.dram_tensor("wch", [6 * NCH, 128, KH * 512], BF16,
                         kind="ExternalInput")
    yr = nc.dram_tensor("yr", [H, cr], BF16, kind="ExternalOutput")
    ys = nc.dram_tensor("ys", [H, CS], BF16, kind="ExternalOutput")

    with tile.TileContext(nc) as tc:
        with tc.tile_pool(name="wpool", bufs=23) as wpool, \
             tc.tile_pool(name="xpool", bufs=1) as xpool, \
             tc.tile_pool(name="hpool", bufs=26) as hpool, \
             tc.tile_pool(name="sgpool", bufs=4) as sgpool, \
             tc.tile_pool(name="ystage", bufs=4) as ystage, \
             tc.tile_pool(name="gupool", bufs=4, space="PSUM") as gupool, \
             tc.tile_pool(name="ypool", bufs=3, space="PSUM") as ypool, \
             tc.tile_pool(name="dpool", bufs=1) as dpool, \
             tc.tile_pool(name="dpsum", bufs=1, space="PSUM") as dpsum:
            # Shared job FIRST: its single 512-col tile demands weight
            # bytes at ~0.14MB/us (vs 0.26 for the 288-col routed tile),
            # well under the ~0.35MB/us HBM supply, so the PE runs at
            # full rate from the first matmul instead of being DMA-paced.
            # One dma_start per x piece (DIRECT2D enqueue is ~590ns each,
            # serialized on the issuing sequencer).
            xr_sb = xpool.tile([128, KH, cr], BF16, tag="xr")
            xs_sb = xpool.tile([128, KH, CS], BF16, tag="xs")

            # The PE powers up in the 1.2GHz mid-pstate and takes ~6us of
            # activity to reach 2.4GHz (first ~14 real matmuls ran at
            # exactly 2x cycle time). Warm the clock with dummy matmuls
            # during the x/weight DMA-wait window so real work starts at
            # full rate.
            dmy = dpool.tile([128, 128], BF16, tag="dmy")
            nc.vector.memset(dmy, 0.0)
            dps = dpsum.tile([128, 128], F32, tag="dps")
            for _ in range(34):
                nc.tensor.matmul(dps, dmy, dmy, start=True, stop=True)

            def x_hook():
                # sync ring, ahead of the weight pieces: the scalar ring's
                # start time is hostage to ACT_TABLE_LOAD jitter. xs rides
                # in two halves; 14 narrow bridge warmers gated only on
                # the first half fire whenever the ring delivers, holding
                # the PE clock through ring-jitter gaps (pstate drops
                # after ~2us idle) at ~0.7us cost instead of the 1.5us a
                # longer fixed burst would add to every core
                nc.sync.dma_start(
                    out=xs_sb[:, 0:3, :],
                    in_=xs[0:384, :].rearrange("(k p) n -> p k n", k=3))
                for i in range(14):
                    nc.tensor.matmul(dps, dmy, xs_sb[:, i % 3, 0:128],
                                     start=True, stop=True)

            def x_hook2():
                nc.sync.dma_start(
                    out=xs_sb[:, 3:KH, :],
                    in_=xs[384:H, :].rearrange("(k p) n -> p k n", k=KH - 3))

            def after_w_hook():
                # bulk x: sync-ring D2D queues behind job1's 14MB of
                # chunks, so the transfer lands ~48us in — off the
                # HBM-saturated ramp, well before its first reader
                nc.sync.dma_start(
                    out=xr_sb,
                    in_=xr.rearrange("(k p) n -> p k n", k=KH))

            _ffn_job(nc, wpool, hpool, sgpool, gupool, ypool, ystage,
                     xs_sb, wch, 3 * NCH, ys, _col_tiles(CS), first=True,
                     x_hook=x_hook, x_hook2=x_hook2,
                     after_w_hook=after_w_hook)
            _ffn_job(nc, wpool, hpool, sgpool, gupool, ypool, ystage,
                     xr_sb, wch, 0, yr, _col_tiles(cr), last=True)
    nc.finalize()
    return nc


def _chunk_gu(wT):
    """[H, I] lhsT-layout weight -> [NCH, 128, 3072] SBUF chunk images.
    j-outer: chunk[c][p, j*768 + k*128 + m] = wT[k*128 + p, (4c+j)*128 + m]"""
    a = wT.reshape(KH, 128, NCH, 4, 128)         # [k, p, c, j, m]
    return np.ascontiguousarray(a.transpose(2, 1, 3, 0, 4)).reshape(NCH, 128, KH * 512)


def _chunk_wd(wdT):
    """[I, H] lhsT-layout down weight -> [NCH, 128, 3072] chunk images.
    chunk[c][p, j*768 + i] = wdT[(4c+j)*128 + p, i]"""
    a = wdT.reshape(NCH, 4, 128, H)              # [c, j, p, i]
    return np.ascontiguousarray(a.transpose(0, 2, 1, 3)).reshape(NCH, 128, 4 * H)


def _pack_chunks(gT, uT, dT):
    """Interleave gate/up/down chunks in consumption order -> [18, 128, 3072]."""
    g = _chunk_gu(gT)
    u = _chunk_gu(uT)
    d = _chunk_wd(dT)
    out = np.empty((3 * NCH, 128, KH * 512), _bf)
    out[0::3] = g
    out[1::3] = u
    out[2::3] = d
    return out


_NC_CACHE = {}


def kernel(hidden_states, gate_w, shared_gate, shared_up, shared_down,
           routed_gate, routed_up, routed_down):
    B, S, _ = hidden_states.shape
    T = B * S
    x = np.asarray(hidden_states, np.float32).reshape(T, H)

    # ---- host router (mirrors reference math; fp64 softmax for stability) --
    logits = x @ np.asarray(gate_w, np.float32).T                    # [T, E]
    lg = logits.astype(np.float64)
    sc = np.exp(lg - lg.max(1, keepdims=True))
    sc /= sc.sum(1, keepdims=True)
    topk_idx = np.argsort(-sc, axis=1, kind="stable")[:, :TOP_K]     # [T, k]
    topk_w = np.take_along_axis(sc, topk_idx, axis=1)
    topk_w = topk_w / (topk_w.sum(1, keepdims=True) + 1e-8)          # [T, k]

    tok_lists = []
    tok_weights = []
    for e in range(E):
        sel = (topk_idx == e)
        toks = np.where(sel.any(1))[0]
        w = (topk_w * sel)[toks].sum(1).astype(np.float32)
        tok_lists.append(toks)
        tok_weights.append(w)
    max_n = max(len(t) for t in tok_lists)
    cr = max(64, -(-max_n // 2) * 2)  # even for half-splits; no 32-pad

    # ---- per-core inputs -------------------------------------------------
    x_bf = x.astype(_bf)
    shared_packs = []
    for s in range(NS):
        sgT = np.ascontiguousarray(np.asarray(shared_gate[s], np.float32).T).astype(_bf)
        suT = np.ascontiguousarray(np.asarray(shared_up[s], np.float32).T).astype(_bf)
        sdT = np.ascontiguousarray(np.asarray(shared_down[s], np.float32).T).astype(_bf)
        shared_packs.append(_pack_chunks(sgT, suT, sdT))

    in_maps = []
    for c in range(N_CORES):
        toks = tok_lists[c]
        xr = np.zeros((H, cr), _bf)
        xr[:, :len(toks)] = x_bf[toks].T
        s = c // (N_CORES // NS)
        q = c % (N_CORES // NS)
        xs_ = np.ascontiguousarray(x_bf[q * CS:(q + 1) * CS].T)
        rgT = np.ascontiguousarray(np.asarray(routed_gate[c], np.float32).T).astype(_bf)
        ruT = np.ascontiguousarray(np.asarray(routed_up[c], np.float32).T).astype(_bf)
        rdT = np.ascontiguousarray(np.asarray(routed_down[c], np.float32).T).astype(_bf)
        wch = np.concatenate([_pack_chunks(rgT, ruT, rdT), shared_packs[s]])
        in_maps.append({"xr": xr, "xs": xs_, "wch": wch})

    # ---- build + run on 8 cores -----------------------------------------
    if cr not in _NC_CACHE:
        _NC_CACHE[cr] = build_nc(cr)
    nc = _NC_CACHE[cr]

    trace = bool(int(os.environ.get("MOE_TRACE", "0")))
    kw = {}
    if trace:
        _install_ntff_hook()
        kw = dict(trace=True, trace_cores=list(range(N_CORES)))
    res = run_bass_kernel_spmd(nc, in_maps, core_ids=list(range(N_CORES)), **kw)
    if trace:
        print(f"HW exec time: {res.exec_time_ns} ns")

    # ---- host combine ----------------------------------------------------
    out = np.zeros((T, H), np.float32)
    for c in range(N_CORES):
        toks = tok_lists[c]
        yrT = res.results[c]["yr"].astype(np.float32)                # [H, cr]
        out[toks] += yrT[:, :len(toks)].T * tok_weights[c][:, None]
        q = c % (N_CORES // NS)
        out[q * CS:(q + 1) * CS] += res.results[c]["ys"].astype(np.float32).T / NS
    return out.reshape(B, S, H)



# revision 47
# speedup vs baseline: 1.1320x; 1.1320x over previous
"""DeepSeek-MoE layer on 8 TRN2 NeuronCores.

Strategy (expert-parallel, host-side dispatch):
  - Router (x @ gate_w.T, softmax, top-2) computed on host — it *is* the
    sharding decision (~0.02% of total FLOPs).
  - Core c computes routed expert c's SwiGLU FFN over the tokens routed to
    it (gathered+padded to a fixed capacity), plus a 512-token chunk of
    shared expert c//4 (each shared expert covers all 2048 tokens, split
    over 4 cores).
  - All matmuls in bf16 (fp32 PSUM accumulation). Combine weights /
    scatter-add applied on host in fp32.

Device kernel layout:
  - Tokens live on the matmul free axis (x stored transposed [H, C]).
  - Weights are streamed as 768KB "4-mi chunks" pre-packed on the host
    into their exact SBUF image (gate/up j-outer), ordered in
    consumption order on the sync HWDGE ring with no phase barriers.
  - The PE powers up in the 1.2GHz mid-pstate and drops back after ~2us
    idle; ~48 dummy 128-col matmuls keep it busy from ~7us so the real
    stream starts at the full 2.4GHz (worth ~3-5us).
  - Shared-expert job runs FIRST: its 512-col tile demands weight bytes
    at ~0.14MB/us vs ~0.35MB/us HBM supply, so the ramp is never
    DMA-paced; the routed job (288+256 col-tiles) follows.
  - Fully fused gate/up loop per 128-row slice mi of I: 6 gate MMs, 6 up
    MMs (PSUM), silu (ACT), mul->bf16 (DVE); down-proj is bank-major (24
    MMs per output bank) so each bank's PSUM->SBUF copy + store overlaps
    the next bank's matmuls and only a half-width copy trails the last
    matmul.
  - dma_start costs ~590ns of serialized DIRECT2D enqueue on the issuing
    sequencer: x tensors ride as single rearranged descriptors (xs ahead
    of the weight pieces, xr queued behind job1's 14MB so it lands ~48us
    in, off the HBM-saturated ramp).
  - Output store copies alternate ACT/DVE half-width, stores alternate
    both HWDGE rings.
"""
import os
import sys
import types

import numpy as np
import ml_dtypes

import concourse.bass as bass
import concourse.tile as tile
import concourse.mybir as mybir
from concourse import bacc
from concourse.bass_utils import run_bass_kernel_spmd

# ---- problem constants (DeepSeekMoE: B=2,S=1024,H=768,I=3072,E=8,NS=2,k=2) --
H = 768          # hidden
I = 3072         # intermediate
E = 8            # routed experts
NS = 2           # shared experts
TOP_K = 2
N_CORES = 8
KH = H // 128    # 6 k-tiles over H
KI = I // 128    # 24 mi-tiles over I
NCH = KI // 4    # 6 weight chunk-groups (4 mi each)
CS = 2048 * NS // N_CORES  # shared-expert tokens per core = 512

BF16 = mybir.dt.bfloat16
F32 = mybir.dt.float32
_bf = ml_dtypes.bfloat16


def _install_ntff_hook():
    """Provide antenv.axon_hooks (missing on this image) so trace=True works."""
    if "antenv.axon_hooks" in sys.modules:
        return
    try:
        from trn_agent_boot.trn_boot import _ntff_profile_via_ctypes
        hook = _ntff_profile_via_ctypes("/opt/axon/libaxon_pjrt.so")
    except Exception:
        hook = None
    mod = types.ModuleType("antenv.axon_hooks")
    mod.get_axon_ntff_profile_hook = lambda: hook
    sys.modules["antenv.axon_hooks"] = mod


def _col_tiles(c):
    if c <= 512:
        return [(0, c)]
    half = (c // 2 + 31) // 32 * 32
    return [(0, half), (half, c - half)]


def _ffn_job(nc, wpool, hpool, sgpool, gupool, ypool, ystage,
             x_sb, wchunks, base, y_dram, n_tiles, first=False,
             x_hook=None, x_hook2=None, after_w_hook=None, last=False):
    """One SwiGLU FFN: y = (silu(x Wg) * (x Wu)) Wd for one expert.

    wchunks[base + 3c + {0,1,2}] are the gate/up/down weight chunks for
    mi-group c, pre-packed on host as the exact [128, 3072] SBUF image
    (gate/up images are j-outer so a single j-slice is contiguous).
    """
    gu_t = {}
    wd_t = {}
    for c in range(NCH):
        row_g = base + 3 * c + 0
        row_u = base + 3 * c + 1
        tg = wpool.tile([128, 4, KH, 128], BF16, tag="w")
        tu = wpool.tile([128, 4, KH, 128], BF16, tag="w")
        if first and c == 0:
            # x first, then g0/u0 split into 192KB j-pieces in
            # consumption order: the first matmul is gated on x plus one
            # 192KB piece instead of a whole 768KB chunk
            if x_hook is not None:
                x_hook()
            for j in range(4):
                nc.sync.dma_start(
                    out=tg[:, j], in_=wchunks[row_g, :, j * 768:(j + 1) * 768]
                    .rearrange("p (k m) -> p k m", k=KH))
                if j == 0 and x_hook2 is not None:
                    x_hook2()
                nc.sync.dma_start(
                    out=tu[:, j], in_=wchunks[row_u, :, j * 768:(j + 1) * 768]
                    .rearrange("p (k m) -> p k m", k=KH))
        else:
            nc.sync.dma_start(out=tg, in_=wchunks[row_g, :, :]
                              .rearrange("p (j k m) -> p j k m", j=4, k=KH))
            nc.sync.dma_start(out=tu, in_=wchunks[row_u, :, :]
                              .rearrange("p (j k m) -> p j k m", j=4, k=KH))
        gu_t[c] = (tg, tu)
    for c in range(NCH):
        td = wpool.tile([128, 4, H], BF16, tag="w")
        nc.sync.dma_start(out=td, in_=wchunks[base + 3 * c + 2, :, :]
                          .rearrange("p (j i) -> p j i", j=4))
        wd_t[c] = td
    if after_w_hook is not None:
        after_w_hook()  # bulk x loads queue here, behind this job's chunks

    for ti, (n0, nsz) in enumerate(n_tiles):
        # gate/up + silu*mul for all 24 mi (4 PSUM banks -> the silu/mul
        # round-trip never stalls the next mi's matmuls)
        hs = {}
        for c in range(NCH):
            tg, tu = gu_t[c]
            for j in range(4):
                g = gupool.tile([128, 512], F32, tag="gu")
                u = gupool.tile([128, 512], F32, tag="gu")
                for k in range(KH):
                    nc.tensor.matmul(
                        g[:, :nsz], tg[:, j, k, :], x_sb[:, k, n0:n0 + nsz],
                        start=(k == 0), stop=(k == KH - 1))
                for k in range(KH):
                    nc.tensor.matmul(
                        u[:, :nsz], tu[:, j, k, :], x_sb[:, k, n0:n0 + nsz],
                        start=(k == 0), stop=(k == KH - 1))
                sg = sgpool.tile([128, 512], F32, tag="sg")
                nc.scalar.activation(sg[:, :nsz], g[:, :nsz],
                                     mybir.ActivationFunctionType.Silu)
                h = hpool.tile([128, 512], BF16, tag="h")
                nc.vector.tensor_mul(h[:, :nsz], sg[:, :nsz], u[:, :nsz])
                hs[4 * c + j] = h
        # down proj bank-major (every bank needs all 24 hs tiles anyway):
        # bank hj's copy+store overlaps bank hj+1's matmuls, so only the
        # final bank's half-width copies trail the last matmul
        for hj in range(KH):
            yb = ypool.tile([128, 512], F32, tag="y", name=f"y{hj}")
            h0 = nsz // 2
            # half-width chains only pay off when each half stays above
            # the ~97ns LDWEIGHTS floor (>=240 cols per half)
            final = (last and ti == len(n_tiles) - 1 and hj == KH - 1
                     and nsz >= 480)
            if final:
                # very last bank: two independent half-width accumulation
                # chains so the first half's copy+store overlaps the
                # second half's matmuls — only a half-width copy trails
                # the kernel's last matmul
                for mi in range(KI):
                    nc.tensor.matmul(
                        yb[:, :h0],
                        wd_t[mi // 4][:, mi % 4, hj * 128:(hj + 1) * 128],
                        hs[mi][:, :h0],
                        start=(mi == 0), stop=(mi == KI - 1))
                for mi in range(KI):
                    nc.tensor.matmul(
                        yb[:, h0:nsz],
                        wd_t[mi // 4][:, mi % 4, hj * 128:(hj + 1) * 128],
                        hs[mi][:, h0:nsz],
                        start=(mi == 0), stop=(mi == KI - 1))
            else:
                for mi in range(KI):
                    nc.tensor.matmul(
                        yb[:, :nsz],
                        wd_t[mi // 4][:, mi % 4, hj * 128:(hj + 1) * 128],
                        hs[mi][:, :nsz],
                        start=(mi == 0), stop=(mi == KI - 1))
            # two half-width copies on ACT+DVE, stores on both HWDGE rings
            yst = ystage.tile([128, 512], BF16, tag="yst")
            nc.scalar.copy(yst[:, :h0], yb[:, :h0])
            nc.vector.tensor_copy(yst[:, h0:nsz], yb[:, h0:nsz])
            nc.scalar.dma_start(
                out=y_dram[hj * 128:(hj + 1) * 128, n0:n0 + h0],
                in_=yst[:, :h0])
            nc.sync.dma_start(
                out=y_dram[hj * 128:(hj + 1) * 128, n0 + h0:n0 + nsz],
                in_=yst[:, h0:nsz])


def build_nc(cr):
    """Build the SPMD program. cr = routed-token capacity (multiple of 32)."""
    nc = bacc.Bacc(None, target_bir_lowering=False)
    xr = nc.dram_tensor("xr", [H, cr], BF16, kind="ExternalInput")
    xs = nc.dram_tensor("xs", [H, CS], BF16, kind="ExternalInput")
    wch = nc.dram_tensor("wch", [6 * NCH, 128, KH * 512], BF16,
                         kind="ExternalInput")
    yr = nc.dram_tensor("yr", [H, cr], BF16, kind="ExternalOutput")
    ys = nc.dram_tensor("ys", [H, CS], BF16, kind="ExternalOutput")

    with tile.TileContext(nc) as tc:
        with tc.tile_pool(name="wpool", bufs=23) as wpool, \
             tc.tile_pool(name="xpool", bufs=1) as xpool, \
             tc.tile_pool(name="hpool", bufs=26) as hpool, \
             tc.tile_pool(name="sgpool", bufs=4) as sgpool, \
             tc.tile_pool(name="ystage", bufs=4) as ystage, \
             tc.tile_pool(name="gupool", bufs=4, space="PSUM") as gupool, \
             tc.tile_pool(name="ypool", bufs=3, space="PSUM") as ypool, \
             tc.tile_pool(name="dpool", bufs=1) as dpool, \
             tc.tile_pool(name="dpsum", bufs=1, space="PSUM") as dpsum:
            # Shared job FIRST: its single 512-col tile demands weight
            # bytes at ~0.14MB/us (vs 0.26 for the 288-col routed tile),
            # well under the ~0.35MB/us HBM supply, so the PE runs at
            # full rate from the first matmul instead of being DMA-paced.
            # One dma_start per x piece (DIRECT2D enqueue is ~590ns each,
            # serialized on the issuing sequencer).
            xr_sb = xpool.tile([128, KH, cr], BF16, tag="xr")
            xs_sb = xpool.tile([128, KH, CS], BF16, tag="xs")

            # The PE powers up in the 1.2GHz mid-pstate and takes ~6us of
            # activity to reach 2.4GHz (first ~14 real matmuls ran at
            # exactly 2x cycle time). Warm the clock with dummy matmuls
            # during the x/weight DMA-wait window so real work starts at
            # full rate.
            dmy = dpool.tile([128, 128], BF16, tag="dmy")
            nc.vector.memset(dmy, 0.0)
            dps = dpsum.tile([128, 128], F32, tag="dps")
            for _ in range(48):
                nc.tensor.matmul(dps, dmy, dmy, start=True, stop=True)

            def x_hook():
                # sync ring, ahead of the weight pieces: the scalar ring's
                # start time is hostage to ACT_TABLE_LOAD jitter
                nc.sync.dma_start(
                    out=xs_sb,
                    in_=xs.rearrange("(k p) n -> p k n", k=KH))

            def after_w_hook():
                # bulk x: sync-ring D2D queues behind job1's 14MB of
                # chunks, so the transfer lands ~48us in — off the
                # HBM-saturated ramp, well before its first reader
                nc.sync.dma_start(
                    out=xr_sb,
                    in_=xr.rearrange("(k p) n -> p k n", k=KH))

            _ffn_job(nc, wpool, hpool, sgpool, gupool, ypool, ystage,
                     xs_sb, wch, 3 * NCH, ys, _col_tiles(CS), first=True,
                     x_hook=x_hook, after_w_hook=after_w_hook)
            _ffn_job(nc, wpool, hpool, sgpool, gupool, ypool, ystage,
                     xr_sb, wch, 0, yr, _col_tiles(cr), last=True)
    nc.finalize()
    return nc


def _chunk_gu(wT):
    """[H, I] lhsT-layout weight -> [NCH, 128, 3072] SBUF chunk images.
    j-outer: chunk[c][p, j*768 + k*128 + m] = wT[k*128 + p, (4c+j)*128 + m]"""
    a = wT.reshape(KH, 128, NCH, 4, 128)         # [k, p, c, j, m]
    return np.ascontiguousarray(a.transpose(2, 1, 3, 0, 4)).reshape(NCH, 128, KH * 512)


def _chunk_wd(wdT):
    """[I, H] lhsT-layout down weight -> [NCH, 128, 3072] chunk images.
    chunk[c][p, j*768 + i] = wdT[(4c+j)*128 + p, i]"""
    a = wdT.reshape(NCH, 4, 128, H)              # [c, j, p, i]
    return np.ascontiguousarray(a.transpose(0, 2, 1, 3)).reshape(NCH, 128, 4 * H)


def _pack_chunks(gT, uT, dT):
    """Interleave gate/up/down chunks in consumption order -> [18, 128, 3072]."""
    g = _chunk_gu(gT)
    u = _chunk_gu(uT)
    d = _chunk_wd(dT)
    out = np.empty((3 * NCH, 128, KH * 512), _bf)
    out[0::3] = g
    out[1::3] = u
    out[2::3] = d
    return out


_NC_CACHE = {}


def kernel(hidden_states, gate_w, shared_gate, shared_up, shared_down,
           routed_gate, routed_up, routed_down):
    B, S, _ = hidden_states.shape
    T = B * S
    x = np.asarray(hidden_states, np.float32).reshape(T, H)

    # ---- host router (mirrors reference math; fp64 softmax for stability) --
    logits = x @ np.asarray(gate_w, np.float32).T                    # [T, E]
    lg = logits.astype(np.float64)
    sc = np.exp(lg - lg.max(1, keepdims=True))
    sc /= sc.sum(1, keepdims=True)
    topk_idx = np.argsort(-sc, axis=1, kind="stable")[:, :TOP_K]     # [T, k]
    topk_w = np.take_along_axis(sc, topk_idx, axis=1)
    topk_w = topk_w / (topk_w.sum(1, keepdims=True) + 1e-8)          # [T, k]

    tok_lists = []
    tok_weights = []
    for e in range(E):
        sel = (topk_idx == e)
        toks = np.where(sel.any(1))[0]
        w = (topk_w * sel)[toks].sum(1).astype(np.float32)
        tok_lists.append(toks)
        tok_weights.append(w)
    max_n = max(len(t) for t in tok_lists)
    cr = max(64, -(-max_n // 2) * 2)  # even for half-splits; no 32-pad

    # ---- per-core inputs -------------------------------------------------
    x_bf = x.astype(_bf)
    shared_packs = []
    for s in range(NS):
        sgT = np.ascontiguousarray(np.asarray(shared_gate[s], np.float32).T).astype(_bf)
        suT = np.ascontiguousarray(np.asarray(shared_up[s], np.float32).T).astype(_bf)
        sdT = np.ascontiguousarray(np.asarray(shared_down[s], np.float32).T).astype(_bf)
        shared_packs.append(_pack_chunks(sgT, suT, sdT))

    in_maps = []
    for c in range(N_CORES):
        toks = tok_lists[c]
        xr = np.zeros((H, cr), _bf)
        xr[:, :len(toks)] = x_bf[toks].T
        s = c // (N_CORES // NS)
        q = c % (N_CORES // NS)
        xs_ = np.ascontiguousarray(x_bf[q * CS:(q + 1) * CS].T)
        rgT = np.ascontiguousarray(np.asarray(routed_gate[c], np.float32).T).astype(_bf)
        ruT = np.ascontiguousarray(np.asarray(routed_up[c], np.float32).T).astype(_bf)
        rdT = np.ascontiguousarray(np.asarray(routed_down[c], np.float32).T).astype(_bf)
        wch = np.concatenate([_pack_chunks(rgT, ruT, rdT), shared_packs[s]])
        in_maps.append({"xr": xr, "xs": xs_, "wch": wch})

    # ---- build + run on 8 cores -----------------------------------------
    if cr not in _NC_CACHE:
        _NC_CACHE[cr] = build_nc(cr)
    nc = _NC_CACHE[cr]

    trace = bool(int(os.environ.get("MOE_TRACE", "0")))
    kw = {}
    if trace:
        _install_ntff_hook()
        kw = dict(trace=True, trace_cores=list(range(N_CORES)))
    res = run_bass_kernel_spmd(nc, in_maps, core_ids=list(range(N_CORES)), **kw)
    if trace:
        print(f"HW exec time: {res.exec_time_ns} ns")

    # ---- host combine ----------------------------------------------------
    out = np.zeros((T, H), np.float32)
    for c in range(N_CORES):
        toks = tok_lists[c]
        yrT = res.results[c]["yr"].astype(np.float32)                # [H, cr]
        out[toks] += yrT[:, :len(toks)].T * tok_weights[c][:, None]
        q = c % (N_CORES // NS)
        out[q * CS:(q + 1) * CS] += res.results[c]["ys"].astype(np.float32).T / NS
    return out.reshape(B, S, H)

